# revision 2
# baseline (speedup 1.0000x reference)
"""Trainium2 Bass kernel for nn_MemoryRNN (T=512, B=51, D=4096, H=1024, R=51).

Structure (verified vs reference): labels are identity -> plain 512-step
LSTM-variant recurrence over [51, 1024] state + one big input GEMM.

v2 changes vs baseline:
  - W_out folded into the recurrence matmul (psB extended to 320 cols), so
    the per-step output head is free (moving dim stays >=256 for fp32r).
  - pin (+ all biases) folded into PSUM via an identity-weights matmul, so
    activations read PSUM directly; removes two vector adds per step.
  - The per-step AllGather of the hidden state is replaced by 7 direct
    SBUF->SBUF remote_dma_broadcast sends (relative (0,dtpb) destinations:
    the DMA ucode XORs with the core's own physical id, so routing never
    needs absolute topology and never leaves the chip). Receivers gate the
    next step's matmuls on the remote semaphore. Double-buffered (parity)
    hT state; flow control is implied by the data dependency chain.
  - The slot<->hidden-block assignment under the relative XOR exchange
    depends on the (unknown) logical->physical core mapping. The kernel
    ships a probe: each core broadcasts its rank once at startup and dumps
    who landed in which slot; the host re-permutes W_s/W_out/h0 blocks and
    reruns if its default XOR-affine assumption was wrong (no recompile).
  - Elementwise restructured: h = hw*(o*tanh(c)) + (px - hw*px) with ops
    spread so the ACT/DVE chains are short; output rows written from psB.
"""
import sys
import os

sys.path.insert(0, '/opt/trn_rl_repo')

import numpy as np

T, B, D, H, R = 512, 51, 4096, 1024, 51
T = int(os.environ.get("K_T", T))  # debug override for fast build tests
NC_ = 8
HS = H // NC_          # 128: per-core hidden shard
GC = 5 * HS            # 640: per-core gate columns (i,f,o,hw,g)
WC = GC + 64           # 704: gate cols + folded W_out block
PC = 832               # pin cols: [ifo 384 | hw,g 256 | zeros 64 | x 128]
KCH = H // 128         # 8 contraction chunks for the recurrence matmul
DCH = D // 128         # 32 contraction chunks for the input GEMM
SAMP = T * B           # 26112
BLK = SAMP // NC_      # samples per pin chunk
STEPS_PER_CHUNK = T // NC_
N_FULL_TILES = BLK // 128
REM_TILE = BLK - 128 * N_FULL_TILES
TILES_PER_CHUNK = N_FULL_TILES + (1 if REM_TILE else 0)

_CACHE = {}


def _np_reference(seq, rel_labels, W_in, b_in, W_s, b_s, W_out, b_out, mem_h0, mem_c0):
    """Pure-numpy fallback, exact reference semantics (handles any labels)."""
    def sigmoid(x):
        return 1.0 / (1.0 + np.exp(-x))
    xs = seq.reshape(T, B, D)
    lbls = (rel_labels.astype(np.int64) % R).reshape(T, B)
    pin_all = xs.astype(np.float32) @ W_in + b_in
    mem_h = mem_h0.copy().astype(np.float32)
    mem_c = mem_c0.copy().astype(np.float32)
    outs = np.empty((T, B, R), np.float32)
    for t in range(T):
        lbl = lbls[t]
        prev_h = mem_h[lbl]
        prev_c = mem_c[lbl]
        ps = prev_h @ W_s + b_s
        pin = pin_all[t]
        i_g = sigmoid(pin[:, 0*H:1*H] + ps[:, 0*H:1*H])
        f_g = sigmoid(pin[:, 1*H:2*H] + ps[:, 1*H:2*H])
        g = np.tanh(pin[:, 2*H:3*H] + ps[:, 2*H:3*H])
        o_g = sigmoid(pin[:, 3*H:4*H] + ps[:, 3*H:4*H])
        c = i_g * g + f_g * prev_c
        h = o_g * np.tanh(c)
        hw = sigmoid(pin[:, 4*H:5*H] + ps[:, 4*H:5*H])
        h = hw * h + (1.0 - hw) * pin[:, 5*H:6*H]
        mem_h[lbl] = h
        mem_c[lbl] = c
        outs[t] = h @ W_out + b_out
    return outs.reshape(T * B, R)


def _labels_are_identity(rel_labels):
    lbls = (np.asarray(rel_labels).astype(np.int64) % R).reshape(T, B)
    return np.array_equal(lbls, np.broadcast_to(np.arange(B, dtype=np.int64), (T, B)))


# ---------------------------------------------------------------------------
# Bass program
# ---------------------------------------------------------------------------

def _build_program():
    import concourse.bacc as bacc
    import concourse.tile as tile
    import concourse.mybir as mybir
    from concourse.tile_rust import add_dep_helper

    f32 = mybir.dt.float32
    f32r = mybir.dt.float32r
    AF = mybir.ActivationFunctionType

    nc = bacc.Bacc("TRN2", target_bir_lowering=False, debug=False, num_devices=NC_)

    # (instruction, sem, value) waits attached AFTER TileContext exits: the
    # tile scheduler's single-core sim cannot satisfy cross-core semaphores
    # (it would deadlock), but at runtime the remote DMA sem increments do.
    post_waits = []

    # ---- I/O ----
    seqT_in = nc.dram_tensor("seqt", [NC_ * D, BLK], f32r, kind="ExternalInput").ap()
    w_in_in = nc.dram_tensor("w_in", [D, 768], f32r, kind="ExternalInput").ap()
    w_s_in = nc.dram_tensor("w_s", [H, WC], f32r, kind="ExternalInput").ap()
    bias_in = nc.dram_tensor("biasb", [128, PC], f32, kind="ExternalInput").ap()
    bout_in = nc.dram_tensor("boutb", [B, R], f32, kind="ExternalInput").ap()
    ident_in = nc.dram_tensor("ident", [B, B], f32, kind="ExternalInput").ap()
    identr_in = nc.dram_tensor("identr", [B, B], f32r, kind="ExternalInput").ap()
    h0t_in = nc.dram_tensor("h0t", [128, KCH, B], f32r, kind="ExternalInput").ap()
    c0_in = nc.dram_tensor("c0", [B, HS], f32, kind="ExternalInput").ap()
    out_dram = nc.dram_tensor("out", [SAMP, R], f32, kind="ExternalOutput").ap()


    with tile.TileContext(nc) as tc:
        with tc.tile_pool(name="const", bufs=1) as constp, \
             tc.tile_pool(name="stream", bufs=2) as streamp, \
             tc.tile_pool(name="pint", bufs=3) as pintp, \
             tc.tile_pool(name="seqs", bufs=3) as seqp, \
             tc.tile_pool(name="psa", bufs=2, space="PSUM") as psa, \
             tc.tile_pool(name="psr", bufs=1, space="PSUM") as psr, \
             tc.tile_pool(name="dram", bufs=1, space="DRAM") as dram:

            # ---- resident constants ----
            w_in_sb = constp.tile([128, DCH, 768], f32r, tag="winsb")
            nc.sync.dma_start(w_in_sb[:], w_in_in.rearrange("(k p) f -> p k f", p=128))
            w_s_sb = constp.tile([128, KCH, WC], f32r, tag="wssb")
            nc.sync.dma_start(w_s_sb[:], w_s_in.rearrange("(k p) f -> p k f", p=128))
            bias_sb = constp.tile([128, PC], f32, tag="biassb")
            nc.sync.dma_start(bias_sb[:], bias_in)
            bout_sb = constp.tile([B, R], f32, tag="boutsb")
            nc.sync.dma_start(bout_sb[:], bout_in)
            ident_sb = constp.tile([B, B], f32, tag="identsb")
            nc.sync.dma_start(ident_sb[:], ident_in)
            identr_sb = constp.tile([B, B], f32r, tag="identrsb")
            nc.sync.dma_start(identr_sb[:], identr_in)

            # persistent hidden-state exchange buffer, double buffered by
            # step parity; slot 0 of each parity is "my" shard.
            hT = constp.tile([128, 2, KCH, 52], f32r, tag="hTbuf")

            # ---- pin chunk DRAM tensors ----
            pc_dram = [dram.tile([BLK, PC], f32r, tag=f"pc{c}", name=f"pc{c}")
                       for c in range(NC_)]

            # ---- phase A in quarter-tile bursts ----
            # one 128-sample tile = 32 k-chunks; a quarter = 8 k-chunks.
            pa_state = {}
            chunk_dmas = {c: [] for c in range(NC_)}

            def emit_phase_a_quarter(c, st, qq):
                m = 128 if st < N_FULL_TILES else REM_TILE
                s0 = 128 * st
                key = (c, st)
                if qq == 0:
                    lhsT = seqp.tile([128, DCH, 128], f32r, tag="seqT")
                    src = seqT_in[D * c:D * (c + 1), s0:s0 + m]
                    nc.sync.dma_start(
                        lhsT[:, :, 0:m], src.rearrange("(k p) s -> p k s", p=128))
                    pA = psa.tile([128, 512], f32, tag="pA")
                    pB = psa.tile([128, 256], f32, tag="pB")
                    pa_state[key] = (lhsT, pA, pB)
                lhsT, pA, pB = pa_state[key]
                for k in range(8 * qq, 8 * qq + 8):
                    lk = lhsT[:, k, 0:m]
                    nc.tensor.matmul(pA[0:m, :], lk,
                                     w_in_sb[:, k, 0:512],
                                     start=(k == 0), stop=(k == DCH - 1))
                    nc.tensor.matmul(pB[0:m, :], lk,
                                     w_in_sb[:, k, 512:768],
                                     start=(k == 0), stop=(k == DCH - 1))
                if qq == 3:
                    pin_sb = streamp.tile([128, PC], f32r, tag="pinsb")
                    # cols [640:704] must be zero (bias cols are zero there)
                    nc.scalar.copy(pin_sb[0:m, 640:704], bias_sb[0:m, 640:704])
                    nc.vector.tensor_add(pin_sb[0:m, 0:512], pA[0:m, :],
                                         bias_sb[0:m, 0:512])
                    nc.vector.tensor_add(pin_sb[0:m, 512:640], pB[0:m, 0:128],
                                         bias_sb[0:m, 512:640])
                    nc.vector.tensor_add(pin_sb[0:m, 704:832], pB[0:m, 128:256],
                                         bias_sb[0:m, 704:832])
                    wdma = nc.sync.dma_start(pc_dram[c][s0:s0 + m, :],
                                             pin_sb[0:m, :])
                    chunk_dmas[c].append((s0, m, wdma))
                    del pa_state[key]

            quarters = [(c, st, q) for c in range(NC_)
                        for st in range(TILES_PER_CHUNK) for q in range(4)]
            q_iter = iter(quarters)

            def emit_quarters(n):
                for _ in range(n):
                    nxt = next(q_iter, None)
                    if nxt is None:
                        return
                    emit_phase_a_quarter(*nxt)

            # first pin chunk(s) up front; tiny-T test mode emits all
            if STEPS_PER_CHUNK < 8:
                emit_quarters(len(quarters))
            else:
                emit_quarters(4 * TILES_PER_CHUNK)

            # ---- initial state ----
            nc.sync.dma_start(hT[:, 1, :, 0:B], h0t_in)
            c_prev = streamp.tile([B, HS], f32, tag="c")
            nc.sync.dma_start(c_prev[:], c0_in)

            # pin prefetch queue
            pin_tiles = {}

            def fetch_pin(t):
                if t >= T:
                    return
                c_idx = t // STEPS_PER_CHUNK
                r0 = B * t - BLK * c_idx
                pt = pintp.tile([B, PC], f32r, tag="pint")
                rdma = nc.sync.dma_start(pt[:], pc_dram[c_idx][r0:r0 + B, :])
                # DRAM-pool tiles are not dep-tracked: order this read after
                # the phase-A writes that cover rows [r0, r0+B)
                for s0, m, wdma in chunk_dmas[c_idx]:
                    if s0 < r0 + B and r0 < s0 + m:
                        add_dep_helper(rdma.ins, wdma.ins, sync=True,
                                       reason="pin chunk RAW")
                pin_tiles[t] = pt

            fetch_pin(0)
            fetch_pin(1)

            # ---- recurrence ----
            for t in range(T):
                p = t & 1          # parity written this step (holds h_t)
                q = (t + 1) & 1    # parity read this step (holds h_{t-1})
                pin_t = pin_tiles.pop(t)
                fetch_pin(t + 2)

                # send preps for THIS step's h_t (desc-gen early, fires at
                # trigger below after h_t is written)
                # psA = pin[ifo] + h @ Ws[ifo]; psB = pin[hw,g]+0 pad, then
                # + h @ [Ws[hw,g] | W_out]
                psA = psr.tile([B, 384], f32, tag="psA")
                psB = psr.tile([B, 320], f32, tag="psB")
                nc.tensor.matmul(psA[:, :], identr_sb[:], pin_t[:, 0:384],
                                 start=True, stop=False)
                nc.tensor.matmul(psB[:, :], identr_sb[:], pin_t[:, 384:704],
                                 start=True, stop=False)

                for k in range(KCH):
                    lh = hT[:, q, k, 0:B]
                    nc.tensor.matmul(psA[:, :], lh,
                                     w_s_sb[:, k, 0:384],
                                     start=False, stop=(k == KCH - 1))
                    nc.tensor.matmul(psB[:, :], lh,
                                     w_s_sb[:, k, 384:704],
                                     start=False, stop=(k == KCH - 1))

                # phase A burst 1: fills the elementwise window
                emit_quarters(1)

                # activations straight off PSUM
                sg_i = streamp.tile([B, HS], f32, tag="sgi")
                nc.scalar.activation(sg_i[:], psA[:, 0:128], AF.Sigmoid)
                gg = streamp.tile([B, HS], f32, tag="gg")
                nc.scalar.activation(gg[:], psB[:, 128:256], AF.Tanh)
                sg_f = streamp.tile([B, HS], f32, tag="sgf")
                nc.scalar.activation(sg_f[:], psA[:, 128:256], AF.Sigmoid)
                sg_hw = streamp.tile([B, HS], f32, tag="sghw")
                nc.scalar.activation(sg_hw[:], psB[:, 0:128], AF.Sigmoid)
                sg_o = streamp.tile([B, HS], f32, tag="sgo")
                nc.scalar.activation(sg_o[:], psA[:, 256:384], AF.Sigmoid)

                cig = streamp.tile([B, HS], f32, tag="cig")
                nc.vector.tensor_mul(cig[:], sg_i[:], gg[:])
                cfc = streamp.tile([B, HS], f32, tag="cfc")
                nc.vector.tensor_mul(cfc[:], sg_f[:], c_prev[:])
                c_new = streamp.tile([B, HS], f32, tag="c")
                nc.vector.tensor_add(c_new[:], cig[:], cfc[:])
                tch = streamp.tile([B, HS], f32, tag="tch")
                nc.scalar.activation(tch[:], c_new[:], AF.Tanh)

                px = pin_t[:, 704:832]
                hwpx = streamp.tile([B, HS], f32, tag="hwpx")
                nc.vector.tensor_mul(hwpx[:], sg_hw[:], px)
                pxm = streamp.tile([B, HS], f32, tag="pxm")
                nc.vector.tensor_sub(pxm[:], px, hwpx[:])
                hl = streamp.tile([B, HS], f32, tag="hl")
                nc.vector.tensor_mul(hl[:], sg_o[:], tch[:])
                hm = streamp.tile([B, HS], f32, tag="hm")
                nc.vector.tensor_mul(hm[:], sg_hw[:], hl[:])
                h_new = streamp.tile([B, HS], f32, tag="h")
                nc.vector.tensor_add(h_new[:], hm[:], pxm[:])

                # transpose h_new -> [HS, B], AllGather into parity p slots
                trp = psr.tile([128, B], f32, tag="trp")
                nc.tensor.transpose(trp[0:HS, :], h_new[:], ident_sb[:])
                hmine = streamp.tile([128, B], f32r, tag="hmine")
                nc.scalar.copy(hmine[0:HS, :], trp[0:HS, :])
                hb = dram.tile([HS, B], f32r, tag=f"hb{t}")
                nc.sync.dma_start(hb[:], hmine[0:HS, :])
                hg = dram.tile([H, B], f32r, addr_space="Shared", tag=f"hg{t}")
                nc.gpsimd.collective_compute(
                    "AllGather", mybir.AluOpType.bypass,
                    replica_groups=[list(range(NC_))],
                    ins=[hb[:]], outs=[hg[:]],
                )
                nc.sync.dma_start(hT[:, p, :, 0:B],
                                  hg[:].rearrange("(k p) b -> p k b", p=128))

                # phase A burst 2: fills the send/wait window
                emit_quarters(1)

                # output head rows for h_{t-1} from the folded W_out cols
                if t >= 1:
                    ob = streamp.tile([B, R], f32, tag="ob")
                    nc.vector.tensor_add(ob[:], psB[:, 256:256 + R], bout_sb[:])
                    nc.sync.dma_start(out_dram[B * (t - 1):B * t, :], ob[:])

                c_prev = c_new

            emit_quarters(len(quarters))  # leftovers (shouldn't be any)

            # final output row block from h_{T-1}
            psF = psr.tile([B, 64], f32, tag="psF")
            for k in range(KCH):
                nc.tensor.matmul(psF[:, :], hT[:, (T + 1) & 1, k, 0:B],
                                 w_s_sb[:, k, 640:704],
                                 start=(k == 0), stop=(k == KCH - 1))
            ob = streamp.tile([B, R], f32, tag="ob")
            nc.vector.tensor_add(ob[:], psF[:, 0:R], bout_sb[:])
            nc.sync.dma_start(out_dram[B * (T - 1):B * T, :], ob[:])

    # attach runtime-only cross-core sem gates (invisible to the scheduler)
    for bi, sem, val in post_waits:
        bi._wait_ge(sem, val)

    nc.compile()
    return nc


# ---------------------------------------------------------------------------
# Host-side sharding / runner
# ---------------------------------------------------------------------------

class _Runner:
    """jit-once SPMD runner via PJRT (mirrors bass2jax.run_bass_via_pjrt)."""

    def __init__(self, nc, replicated_names=()):
        import jax
        from jax.experimental.shard_map import shard_map
        from jax.sharding import Mesh, NamedSharding, PartitionSpec
        import concourse.mybir as mybir
        from concourse.bass2jax import (_bass_exec_p, install_neuronx_cc_hook,
                                        partition_id_tensor)
        self.jax = jax
        install_neuronx_cc_hook()
        self.nc = nc
        self.n_cores = NC_
        self.replicated = set(replicated_names)
        partition_name = nc.partition_id_tensor.name if nc.partition_id_tensor else None
        in_names, out_names, out_avals, zero_outs = [], [], [], []
        for alloc in nc.m.functions[0].allocations:
            if not isinstance(alloc, mybir.MemoryLocationSet):
                continue
            name = alloc.memorylocations[0].name
            if alloc.kind == "ExternalInput":
                if name != partition_name:
                    in_names.append(name)
            elif alloc.kind == "ExternalOutput":
                shape = tuple(alloc.tensor_shape)
                dtype = mybir.dt.np(alloc.dtype)
                out_names.append(name)
                out_avals.append(jax.core.ShapedArray(shape, dtype))
                zero_outs.append(np.zeros(shape, dtype))
        self.in_names, self.out_names = in_names, out_names
        self.out_avals, self.zero_outs = out_avals, zero_outs
        n_params, n_outs = len(in_names), len(out_names)
        all_in = list(in_names) + list(out_names)
        if partition_name is not None:
            all_in.append(partition_name)

        def _body(*args):
            operands = list(args)
            if partition_name is not None:
                operands.append(partition_id_tensor())
            outs = _bass_exec_p.bind(
                *operands,
                out_avals=tuple(out_avals),
                in_names=tuple(all_in),
                out_names=tuple(out_names),
                lowering_input_output_aliases=(),
                sim_require_finite=True,
                sim_require_nnan=True,
                nc=nc,
            )
            return tuple(outs)

        devices = jax.devices()[:NC_]
        self.mesh = Mesh(np.asarray(devices), ("core",))
        P = PartitionSpec
        in_specs = tuple(
            P(None) if name in self.replicated else P("core") for name in in_names
        ) + (P("core"),) * n_outs
        out_specs = (P("core"),) * n_outs
        self.sharded = jax.jit(
            shard_map(_body, mesh=self.mesh, in_specs=in_specs,
                      out_specs=out_specs, check_rep=False),
            keep_unused=True,
        )
        self.shard_spec = NamedSharding(self.mesh, P("core"))
        self.repl_spec = NamedSharding(self.mesh, P(None))

    def stage(self, in_maps):
        jax = self.jax
        args = []
        for i, name in enumerate(self.in_names):
            if name in self.replicated:
                args.append(jax.device_put(np.asarray(in_maps[0][name]),
                                           self.repl_spec))
            else:
                cat = np.concatenate(
                    [np.asarray(m[name]) for m in in_maps], axis=0)
                args.append(jax.device_put(cat, self.shard_spec))
        for z in self.zero_outs:
            cat = np.zeros((self.n_cores * z.shape[0], *z.shape[1:]), z.dtype)
            args.append(jax.device_put(cat, self.shard_spec))
        return args

    def run(self, args):
        outs = self.sharded(*args)
        self.jax.block_until_ready(outs)
        return outs

    def results(self, outs, core=0):
        res = {}
        for i, name in enumerate(self.out_names):
            a = np.asarray(outs[i])
            res[name] = a.reshape(self.n_cores, *self.out_avals[i].shape)[core]
        return res

    def results_all(self, outs, name):
        i = self.out_names.index(name)
        a = np.asarray(outs[i])
        return a.reshape(self.n_cores, *self.out_avals[i].shape)


def _prep_inputs(seq, W_in, b_in, W_s, b_s, W_out, b_out, mem_h0, mem_c0,
                 slot_map=None):
    """Host-side sharding/layout prep. Returns per-core in_maps.

    slot_map[j][kappa] = logical rank whose h-block lands in slot kappa on
    core j under the relative-XOR exchange. Default: j ^ kappa.
    """
    seq = np.asarray(seq, np.float32)
    W_in = np.asarray(W_in, np.float32)
    b_in = np.asarray(b_in, np.float32)
    W_s = np.asarray(W_s, np.float32)
    b_s = np.asarray(b_s, np.float32)
    W_out = np.asarray(W_out, np.float32)
    b_out = np.asarray(b_out, np.float32)
    mem_h0 = np.asarray(mem_h0, np.float32)
    mem_c0 = np.asarray(mem_c0, np.float32)

    if slot_map is None:
        slot_map = [[k for k in range(NC_)] for j in range(NC_)]

    seqT = np.ascontiguousarray(seq.T)               # [D, SAMP]
    seqT_blocked = np.concatenate(
        [seqT[:, BLK * c:BLK * (c + 1)] for c in range(NC_)], axis=0)

    # reference gate order: i, f, g, o, hw, x -> ours: i, f, o, hw, g, x
    order6 = [0, 1, 3, 4, 2, 5]
    order5 = [0, 1, 3, 4, 2]
    in_maps = []
    bs_eff = b_in[:5 * H] + b_s
    w_out_pad = np.pad(W_out, ((0, 0), (0, 64 - R)))  # [H, 64]
    for j in range(NC_):
        sl = slice(HS * j, HS * (j + 1))
        w_in_j = np.concatenate(
            [W_in[:, H * g:H * (g + 1)][:, sl] for g in order6], axis=1)
        w_s_j = np.concatenate(
            [W_s[:, H * g:H * (g + 1)][:, sl] for g in order5]
            + [w_out_pad], axis=1)                    # [H, 704]
        # permute contraction row-blocks to match exchange slot order
        w_s_jp = np.concatenate(
            [w_s_j[128 * slot_map[j][k]:128 * (slot_map[j][k] + 1), :]
             for k in range(NC_)], axis=0)
        bias_j = np.concatenate(
            [bs_eff[H * g:H * (g + 1)][sl] for g in order5]
            + [np.zeros(64, np.float32)]
            + [b_in[5 * H:6 * H][sl]])                # [832]
        bias_b = np.broadcast_to(bias_j, (128, PC)).copy()
        bout_b = np.broadcast_to(b_out, (B, R)).copy()
        ident = np.eye(B, dtype=np.float32)
        # h0t[p, k, b] = mem_h0[b, 128*slot_map[j][k] + p]
        h0t = np.transpose(mem_h0.T.reshape(KCH, 128, B), (1, 0, 2))
        h0t_p = np.ascontiguousarray(h0t[:, slot_map[j], :])
        in_maps.append({
            "seqt": seqT_blocked,
            "w_in": np.ascontiguousarray(w_in_j),
            "w_s": np.ascontiguousarray(w_s_jp),
            "biasb": np.ascontiguousarray(bias_b),
            "boutb": np.ascontiguousarray(bout_b),
            "ident": ident,
            "identr": ident.copy(),
            "h0t": h0t_p,
            "c0": np.ascontiguousarray(mem_c0[:, sl]),
        })
    return in_maps


def get_runner():
    if "runner" not in _CACHE:
        nc = _build_program()
        _CACHE["runner"] = _Runner(nc, replicated_names={"seqt"})
    return _CACHE["runner"]


def _slot_map_ok(pmaps):
    """pmaps: [8, 8, 4] per-core probe dumps. Returns (ok, learned_map)."""
    learned = []
    for j in range(NC_):
        row = [int(round(float(pmaps[j][k][0]))) for k in range(NC_)]
        learned.append(row)
    default = [[j ^ k for k in range(NC_)] for j in range(NC_)]
    return learned == default, learned


def kernel(seq, rel_labels, W_in, b_in, W_s, b_s, W_out, b_out, mem_h0, mem_c0):
    if not _labels_are_identity(rel_labels):
        return _np_reference(seq, rel_labels, W_in, b_in, W_s, b_s,
                             W_out, b_out, mem_h0, mem_c0)
    r = get_runner()
    in_maps = _prep_inputs(seq, W_in, b_in, W_s, b_s, W_out, b_out,
                           mem_h0, mem_c0)
    args = r.stage(in_maps)
    outs = r.run(args)
    return r.results(outs, core=0)["out"]


# revision 3
# speedup vs baseline: 1.0186x; 1.0186x over previous
"""Trainium2 Bass kernel for nn_MemoryRNN (T=512, B=51, D=4096, H=1024, R=51).

Structure (verified vs reference): labels are identity -> plain 512-step
LSTM-variant recurrence over [51, 1024] state + one big input GEMM.

Changes vs the earlier baseline (same 8-way tensor-parallel layout, same
per-step AllGather of the hidden state through shared DRAM):
  - W_out folded into the recurrence matmul (psB extended to 320 cols), so
    the per-step output head is free (moving dim stays >=256 for fp32r).
  - pin (+ all biases) folded into PSUM via an identity-weights matmul, so
    activations read PSUM directly; removes two vector adds per step.
  - Elementwise restructured: h = hw*(o*tanh(c)) + (px - hw*px) with ops
    spread so the ACT/DVE chains are short; output rows written from psB.
  - Double-buffered (parity) persistent hT exchange buffer; pin prefetched
    two steps ahead; phase-A input GEMM emitted in quarter-tile bursts that
    the scheduler interleaves into the recurrence's collective-wait gaps.
  - Explicit RAW deps on the pin-chunk DRAM staging (DRAM-pool tiles are
    not dependency-tracked by the Tile framework).
"""
import sys
import os

sys.path.insert(0, '/opt/trn_rl_repo')

import numpy as np

T, B, D, H, R = 512, 51, 4096, 1024, 51
T = int(os.environ.get("K_T", T))  # debug override for fast build tests
NC_ = 8
HS = H // NC_          # 128: per-core hidden shard
GC = 5 * HS            # 640: per-core gate columns (i,f,o,hw,g)
WC = GC + 64           # 704: gate cols + folded W_out block
PC = 832               # pin cols: [ifo 384 | hw,g 256 | zeros 64 | x 128]
KCH = H // 128         # 8 contraction chunks for the recurrence matmul
DCH = D // 128         # 32 contraction chunks for the input GEMM
SAMP = T * B           # 26112
BLK = SAMP // NC_      # samples per pin chunk
STEPS_PER_CHUNK = T // NC_
N_FULL_TILES = BLK // 128
REM_TILE = BLK - 128 * N_FULL_TILES
TILES_PER_CHUNK = N_FULL_TILES + (1 if REM_TILE else 0)

_CACHE = {}


def _np_reference(seq, rel_labels, W_in, b_in, W_s, b_s, W_out, b_out, mem_h0, mem_c0):
    """Pure-numpy fallback, exact reference semantics (handles any labels)."""
    def sigmoid(x):
        return 1.0 / (1.0 + np.exp(-x))
    xs = seq.reshape(T, B, D)
    lbls = (rel_labels.astype(np.int64) % R).reshape(T, B)
    pin_all = xs.astype(np.float32) @ W_in + b_in
    mem_h = mem_h0.copy().astype(np.float32)
    mem_c = mem_c0.copy().astype(np.float32)
    outs = np.empty((T, B, R), np.float32)
    for t in range(T):
        lbl = lbls[t]
        prev_h = mem_h[lbl]
        prev_c = mem_c[lbl]
        ps = prev_h @ W_s + b_s
        pin = pin_all[t]
        i_g = sigmoid(pin[:, 0*H:1*H] + ps[:, 0*H:1*H])
        f_g = sigmoid(pin[:, 1*H:2*H] + ps[:, 1*H:2*H])
        g = np.tanh(pin[:, 2*H:3*H] + ps[:, 2*H:3*H])
        o_g = sigmoid(pin[:, 3*H:4*H] + ps[:, 3*H:4*H])
        c = i_g * g + f_g * prev_c
        h = o_g * np.tanh(c)
        hw = sigmoid(pin[:, 4*H:5*H] + ps[:, 4*H:5*H])
        h = hw * h + (1.0 - hw) * pin[:, 5*H:6*H]
        mem_h[lbl] = h
        mem_c[lbl] = c
        outs[t] = h @ W_out + b_out
    return outs.reshape(T * B, R)


def _labels_are_identity(rel_labels):
    lbls = (np.asarray(rel_labels).astype(np.int64) % R).reshape(T, B)
    return np.array_equal(lbls, np.broadcast_to(np.arange(B, dtype=np.int64), (T, B)))


# ---------------------------------------------------------------------------
# Bass program
# ---------------------------------------------------------------------------

def _build_program():
    import concourse.bacc as bacc
    import concourse.tile as tile
    import concourse.mybir as mybir
    from concourse.tile_rust import add_dep_helper

    f32 = mybir.dt.float32
    f32r = mybir.dt.float32r
    AF = mybir.ActivationFunctionType

    nc = bacc.Bacc("TRN2", target_bir_lowering=False, debug=False, num_devices=NC_)

    # (instruction, sem, value) waits attached AFTER TileContext exits: the
    # tile scheduler's single-core sim cannot satisfy cross-core semaphores
    # (it would deadlock), but at runtime the remote DMA sem increments do.
    post_waits = []

    # ---- I/O ----
    seqT_in = nc.dram_tensor("seqt", [NC_ * D, BLK], f32r, kind="ExternalInput").ap()
    w_in_in = nc.dram_tensor("w_in", [D, 768], f32r, kind="ExternalInput").ap()
    w_s_in = nc.dram_tensor("w_s", [H, WC], f32r, kind="ExternalInput").ap()
    bias_in = nc.dram_tensor("biasb", [128, PC], f32, kind="ExternalInput").ap()
    bout_in = nc.dram_tensor("boutb", [B, R], f32, kind="ExternalInput").ap()
    ident_in = nc.dram_tensor("ident", [B, B], f32, kind="ExternalInput").ap()
    identr_in = nc.dram_tensor("identr", [B, B], f32r, kind="ExternalInput").ap()
    h0t_in = nc.dram_tensor("h0t", [128, KCH, B], f32r, kind="ExternalInput").ap()
    c0_in = nc.dram_tensor("c0", [B, HS], f32, kind="ExternalInput").ap()
    out_dram = nc.dram_tensor("out", [SAMP, R], f32, kind="ExternalOutput").ap()


    with tile.TileContext(nc) as tc:
        with tc.tile_pool(name="const", bufs=1) as constp, \
             tc.tile_pool(name="stream", bufs=2) as streamp, \
             tc.tile_pool(name="pint", bufs=3) as pintp, \
             tc.tile_pool(name="seqs", bufs=3) as seqp, \
             tc.tile_pool(name="psa", bufs=2, space="PSUM") as psa, \
             tc.tile_pool(name="psr", bufs=1, space="PSUM") as psr, \
             tc.tile_pool(name="dram", bufs=1, space="DRAM") as dram:

            # ---- resident constants ----
            w_in_sb = constp.tile([128, DCH, 768], f32r, tag="winsb")
            nc.sync.dma_start(w_in_sb[:], w_in_in.rearrange("(k p) f -> p k f", p=128))
            w_s_sb = constp.tile([128, KCH, WC], f32r, tag="wssb")
            nc.sync.dma_start(w_s_sb[:], w_s_in.rearrange("(k p) f -> p k f", p=128))
            bias_sb = constp.tile([128, PC], f32, tag="biassb")
            nc.sync.dma_start(bias_sb[:], bias_in)
            bout_sb = constp.tile([B, R], f32, tag="boutsb")
            nc.sync.dma_start(bout_sb[:], bout_in)
            ident_sb = constp.tile([B, B], f32, tag="identsb")
            nc.sync.dma_start(ident_sb[:], ident_in)
            identr_sb = constp.tile([B, B], f32r, tag="identrsb")
            nc.sync.dma_start(identr_sb[:], identr_in)

            # persistent hidden-state exchange buffer, double buffered by
            # step parity; slot 0 of each parity is "my" shard.
            hT = constp.tile([128, 2, KCH, 52], f32r, tag="hTbuf")

            # ---- pin chunk DRAM tensors ----
            pc_dram = [dram.tile([BLK, PC], f32r, tag=f"pc{c}", name=f"pc{c}")
                       for c in range(NC_)]

            # ---- phase A in quarter-tile bursts ----
            # one 128-sample tile = 32 k-chunks; a quarter = 8 k-chunks.
            pa_state = {}
            chunk_dmas = {c: [] for c in range(NC_)}

            def emit_phase_a_quarter(c, st, qq):
                m = 128 if st < N_FULL_TILES else REM_TILE
                s0 = 128 * st
                key = (c, st)
                if qq == 0:
                    lhsT = seqp.tile([128, DCH, 128], f32r, tag="seqT")
                    src = seqT_in[D * c:D * (c + 1), s0:s0 + m]
                    nc.sync.dma_start(
                        lhsT[:, :, 0:m], src.rearrange("(k p) s -> p k s", p=128))
                    pA = psa.tile([128, 512], f32, tag="pA")
                    pB = psa.tile([128, 256], f32, tag="pB")
                    pa_state[key] = (lhsT, pA, pB)
                lhsT, pA, pB = pa_state[key]
                for k in range(8 * qq, 8 * qq + 8):
                    lk = lhsT[:, k, 0:m]
                    nc.tensor.matmul(pA[0:m, :], lk,
                                     w_in_sb[:, k, 0:512],
                                     start=(k == 0), stop=(k == DCH - 1))
                    nc.tensor.matmul(pB[0:m, :], lk,
                                     w_in_sb[:, k, 512:768],
                                     start=(k == 0), stop=(k == DCH - 1))
                if qq == 3:
                    pin_sb = streamp.tile([128, PC], f32r, tag="pinsb")
                    # cols [640:704] must be zero (bias cols are zero there)
                    nc.scalar.copy(pin_sb[0:m, 640:704], bias_sb[0:m, 640:704])
                    nc.vector.tensor_add(pin_sb[0:m, 0:512], pA[0:m, :],
                                         bias_sb[0:m, 0:512])
                    nc.vector.tensor_add(pin_sb[0:m, 512:640], pB[0:m, 0:128],
                                         bias_sb[0:m, 512:640])
                    nc.vector.tensor_add(pin_sb[0:m, 704:832], pB[0:m, 128:256],
                                         bias_sb[0:m, 704:832])
                    wdma = nc.sync.dma_start(pc_dram[c][s0:s0 + m, :],
                                             pin_sb[0:m, :])
                    chunk_dmas[c].append((s0, m, wdma))
                    del pa_state[key]

            quarters = [(c, st, q) for c in range(NC_)
                        for st in range(TILES_PER_CHUNK) for q in range(4)]
            q_iter = iter(quarters)

            def emit_quarters(n):
                for _ in range(n):
                    nxt = next(q_iter, None)
                    if nxt is None:
                        return
                    emit_phase_a_quarter(*nxt)

            # first pin chunk(s) up front; tiny-T test mode emits all
            if STEPS_PER_CHUNK < 8:
                emit_quarters(len(quarters))
            else:
                emit_quarters(4 * TILES_PER_CHUNK)

            # ---- initial state ----
            nc.sync.dma_start(hT[:, 1, :, 0:B], h0t_in)
            c_prev = streamp.tile([B, HS], f32, tag="c")
            nc.sync.dma_start(c_prev[:], c0_in)

            # pin prefetch queue
            pin_tiles = {}

            def fetch_pin(t):
                if t >= T:
                    return
                c_idx = t // STEPS_PER_CHUNK
                r0 = B * t - BLK * c_idx
                pt = pintp.tile([B, PC], f32r, tag="pint")
                rdma = nc.sync.dma_start(pt[:], pc_dram[c_idx][r0:r0 + B, :])
                # DRAM-pool tiles are not dep-tracked: order this read after
                # the phase-A writes that cover rows [r0, r0+B)
                for s0, m, wdma in chunk_dmas[c_idx]:
                    if s0 < r0 + B and r0 < s0 + m:
                        add_dep_helper(rdma.ins, wdma.ins, sync=True,
                                       reason="pin chunk RAW")
                pin_tiles[t] = pt

            fetch_pin(0)
            fetch_pin(1)

            # ---- recurrence ----
            for t in range(T):
                p = t & 1          # parity written this step (holds h_t)
                q = (t + 1) & 1    # parity read this step (holds h_{t-1})
                pin_t = pin_tiles.pop(t)
                fetch_pin(t + 2)

                # send preps for THIS step's h_t (desc-gen early, fires at
                # trigger below after h_t is written)
                # psA = pin[ifo] + h @ Ws[ifo]; psB = pin[hw,g]+0 pad, then
                # + h @ [Ws[hw,g] | W_out]
                psA = psr.tile([B, 384], f32, tag="psA")
                psB = psr.tile([B, 320], f32, tag="psB")
                nc.tensor.matmul(psA[:, :], identr_sb[:], pin_t[:, 0:384],
                                 start=True, stop=False)
                nc.tensor.matmul(psB[:, :], identr_sb[:], pin_t[:, 384:704],
                                 start=True, stop=False)

                for k in range(KCH):
                    lh = hT[:, q, k, 0:B]
                    nc.tensor.matmul(psA[:, :], lh,
                                     w_s_sb[:, k, 0:384],
                                     start=False, stop=(k == KCH - 1))
                    nc.tensor.matmul(psB[:, :], lh,
                                     w_s_sb[:, k, 384:704],
                                     start=False, stop=(k == KCH - 1))

                # phase A burst 1: fills the elementwise window
                emit_quarters(1)

                # activations straight off PSUM
                sg_i = streamp.tile([B, HS], f32, tag="sgi")
                nc.scalar.activation(sg_i[:], psA[:, 0:128], AF.Sigmoid)
                gg = streamp.tile([B, HS], f32, tag="gg")
                nc.scalar.activation(gg[:], psB[:, 128:256], AF.Tanh)
                sg_f = streamp.tile([B, HS], f32, tag="sgf")
                nc.scalar.activation(sg_f[:], psA[:, 128:256], AF.Sigmoid)
                sg_hw = streamp.tile([B, HS], f32, tag="sghw")
                nc.scalar.activation(sg_hw[:], psB[:, 0:128], AF.Sigmoid)
                sg_o = streamp.tile([B, HS], f32, tag="sgo")
                nc.scalar.activation(sg_o[:], psA[:, 256:384], AF.Sigmoid)

                cig = streamp.tile([B, HS], f32, tag="cig")
                nc.vector.tensor_mul(cig[:], sg_i[:], gg[:])
                cfc = streamp.tile([B, HS], f32, tag="cfc")
                nc.vector.tensor_mul(cfc[:], sg_f[:], c_prev[:])
                c_new = streamp.tile([B, HS], f32, tag="c")
                nc.vector.tensor_add(c_new[:], cig[:], cfc[:])
                tch = streamp.tile([B, HS], f32, tag="tch")
                nc.scalar.activation(tch[:], c_new[:], AF.Tanh)

                px = pin_t[:, 704:832]
                hwpx = streamp.tile([B, HS], f32, tag="hwpx")
                nc.vector.tensor_mul(hwpx[:], sg_hw[:], px)
                pxm = streamp.tile([B, HS], f32, tag="pxm")
                nc.vector.tensor_sub(pxm[:], px, hwpx[:])
                hl = streamp.tile([B, HS], f32, tag="hl")
                nc.vector.tensor_mul(hl[:], sg_o[:], tch[:])
                hm = streamp.tile([B, HS], f32, tag="hm")
                nc.vector.tensor_mul(hm[:], sg_hw[:], hl[:])
                h_new = streamp.tile([B, HS], f32, tag="h")
                nc.vector.tensor_add(h_new[:], hm[:], pxm[:])

                # transpose h_new -> [HS, B], AllGather into parity p slots
                trp = psr.tile([128, B], f32, tag="trp")
                nc.tensor.transpose(trp[0:HS, :], h_new[:], ident_sb[:])
                hmine = streamp.tile([128, B], f32r, tag="hmine")
                nc.scalar.copy(hmine[0:HS, :], trp[0:HS, :])
                hb = dram.tile([HS, B], f32r, tag=f"hb{t}")
                nc.sync.dma_start(hb[:], hmine[0:HS, :])
                hg = dram.tile([H, B], f32r, addr_space="Shared", tag=f"hg{t}")
                nc.gpsimd.collective_compute(
                    "AllGather", mybir.AluOpType.bypass,
                    replica_groups=[list(range(NC_))],
                    ins=[hb[:]], outs=[hg[:]],
                )
                nc.sync.dma_start(hT[:, p, :, 0:B],
                                  hg[:].rearrange("(k p) b -> p k b", p=128))

                # phase A burst 2: fills the send/wait window
                emit_quarters(1)

                # output head rows for h_{t-1} from the folded W_out cols
                if t >= 1:
                    ob = streamp.tile([B, R], f32, tag="ob")
                    nc.vector.tensor_add(ob[:], psB[:, 256:256 + R], bout_sb[:])
                    nc.sync.dma_start(out_dram[B * (t - 1):B * t, :], ob[:])

                c_prev = c_new

            emit_quarters(len(quarters))  # leftovers (shouldn't be any)

            # final output row block from h_{T-1}
            psF = psr.tile([B, 64], f32, tag="psF")
            for k in range(KCH):
                nc.tensor.matmul(psF[:, :], hT[:, (T + 1) & 1, k, 0:B],
                                 w_s_sb[:, k, 640:704],
                                 start=(k == 0), stop=(k == KCH - 1))
            ob = streamp.tile([B, R], f32, tag="ob")
            nc.vector.tensor_add(ob[:], psF[:, 0:R], bout_sb[:])
            nc.sync.dma_start(out_dram[B * (T - 1):B * T, :], ob[:])

    # attach runtime-only cross-core sem gates (invisible to the scheduler)
    for bi, sem, val in post_waits:
        bi._wait_ge(sem, val)

    nc.compile()
    return nc


# ---------------------------------------------------------------------------
# Host-side sharding / runner
# ---------------------------------------------------------------------------

class _Runner:
    """jit-once SPMD runner via PJRT (mirrors bass2jax.run_bass_via_pjrt)."""

    def __init__(self, nc, replicated_names=()):
        import jax
        from jax.experimental.shard_map import shard_map
        from jax.sharding import Mesh, NamedSharding, PartitionSpec
        import concourse.mybir as mybir
        from concourse.bass2jax import (_bass_exec_p, install_neuronx_cc_hook,
                                        partition_id_tensor)
        self.jax = jax
        install_neuronx_cc_hook()
        self.nc = nc
        self.n_cores = NC_
        self.replicated = set(replicated_names)
        partition_name = nc.partition_id_tensor.name if nc.partition_id_tensor else None
        in_names, out_names, out_avals, zero_outs = [], [], [], []
        for alloc in nc.m.functions[0].allocations:
            if not isinstance(alloc, mybir.MemoryLocationSet):
                continue
            name = alloc.memorylocations[0].name
            if alloc.kind == "ExternalInput":
                if name != partition_name:
                    in_names.append(name)
            elif alloc.kind == "ExternalOutput":
                shape = tuple(alloc.tensor_shape)
                dtype = mybir.dt.np(alloc.dtype)
                out_names.append(name)
                out_avals.append(jax.core.ShapedArray(shape, dtype))
                zero_outs.append(np.zeros(shape, dtype))
        self.in_names, self.out_names = in_names, out_names
        self.out_avals, self.zero_outs = out_avals, zero_outs
        n_params, n_outs = len(in_names), len(out_names)
        all_in = list(in_names) + list(out_names)
        if partition_name is not None:
            all_in.append(partition_name)

        def _body(*args):
            operands = list(args)
            if partition_name is not None:
                operands.append(partition_id_tensor())
            outs = _bass_exec_p.bind(
                *operands,
                out_avals=tuple(out_avals),
                in_names=tuple(all_in),
                out_names=tuple(out_names),
                lowering_input_output_aliases=(),
                sim_require_finite=True,
                sim_require_nnan=True,
                nc=nc,
            )
            return tuple(outs)

        devices = jax.devices()[:NC_]
        self.mesh = Mesh(np.asarray(devices), ("core",))
        P = PartitionSpec
        in_specs = tuple(
            P(None) if name in self.replicated else P("core") for name in in_names
        ) + (P("core"),) * n_outs
        out_specs = (P("core"),) * n_outs
        self.sharded = jax.jit(
            shard_map(_body, mesh=self.mesh, in_specs=in_specs,
                      out_specs=out_specs, check_rep=False),
            keep_unused=True,
        )
        self.shard_spec = NamedSharding(self.mesh, P("core"))
        self.repl_spec = NamedSharding(self.mesh, P(None))

    def stage(self, in_maps):
        jax = self.jax
        args = []
        for i, name in enumerate(self.in_names):
            if name in self.replicated:
                args.append(jax.device_put(np.asarray(in_maps[0][name]),
                                           self.repl_spec))
            else:
                cat = np.concatenate(
                    [np.asarray(m[name]) for m in in_maps], axis=0)
                args.append(jax.device_put(cat, self.shard_spec))
        for z in self.zero_outs:
            cat = np.zeros((self.n_cores * z.shape[0], *z.shape[1:]), z.dtype)
            args.append(jax.device_put(cat, self.shard_spec))
        return args

    def run(self, args):
        outs = self.sharded(*args)
        self.jax.block_until_ready(outs)
        return outs

    def results(self, outs, core=0):
        res = {}
        for i, name in enumerate(self.out_names):
            a = np.asarray(outs[i])
            res[name] = a.reshape(self.n_cores, *self.out_avals[i].shape)[core]
        return res

    def results_all(self, outs, name):
        i = self.out_names.index(name)
        a = np.asarray(outs[i])
        return a.reshape(self.n_cores, *self.out_avals[i].shape)


def _prep_inputs(seq, W_in, b_in, W_s, b_s, W_out, b_out, mem_h0, mem_c0,
                 slot_map=None):
    """Host-side sharding/layout prep. Returns per-core in_maps.

    slot_map[j][kappa] = logical rank whose h-block lands in slot kappa on
    core j under the relative-XOR exchange. Default: j ^ kappa.
    """
    seq = np.asarray(seq, np.float32)
    W_in = np.asarray(W_in, np.float32)
    b_in = np.asarray(b_in, np.float32)
    W_s = np.asarray(W_s, np.float32)
    b_s = np.asarray(b_s, np.float32)
    W_out = np.asarray(W_out, np.float32)
    b_out = np.asarray(b_out, np.float32)
    mem_h0 = np.asarray(mem_h0, np.float32)
    mem_c0 = np.asarray(mem_c0, np.float32)

    if slot_map is None:
        slot_map = [[k for k in range(NC_)] for j in range(NC_)]

    seqT = np.ascontiguousarray(seq.T)               # [D, SAMP]
    seqT_blocked = np.concatenate(
        [seqT[:, BLK * c:BLK * (c + 1)] for c in range(NC_)], axis=0)

    # reference gate order: i, f, g, o, hw, x -> ours: i, f, o, hw, g, x
    order6 = [0, 1, 3, 4, 2, 5]
    order5 = [0, 1, 3, 4, 2]
    in_maps = []
    bs_eff = b_in[:5 * H] + b_s
    w_out_pad = np.pad(W_out, ((0, 0), (0, 64 - R)))  # [H, 64]
    for j in range(NC_):
        sl = slice(HS * j, HS * (j + 1))
        w_in_j = np.concatenate(
            [W_in[:, H * g:H * (g + 1)][:, sl] for g in order6], axis=1)
        w_s_j = np.concatenate(
            [W_s[:, H * g:H * (g + 1)][:, sl] for g in order5]
            + [w_out_pad], axis=1)                    # [H, 704]
        # permute contraction row-blocks to match exchange slot order
        w_s_jp = np.concatenate(
            [w_s_j[128 * slot_map[j][k]:128 * (slot_map[j][k] + 1), :]
             for k in range(NC_)], axis=0)
        bias_j = np.concatenate(
            [bs_eff[H * g:H * (g + 1)][sl] for g in order5]
            + [np.zeros(64, np.float32)]
            + [b_in[5 * H:6 * H][sl]])                # [832]
        bias_b = np.broadcast_to(bias_j, (128, PC)).copy()
        bout_b = np.broadcast_to(b_out, (B, R)).copy()
        ident = np.eye(B, dtype=np.float32)
        # h0t[p, k, b] = mem_h0[b, 128*slot_map[j][k] + p]
        h0t = np.transpose(mem_h0.T.reshape(KCH, 128, B), (1, 0, 2))
        h0t_p = np.ascontiguousarray(h0t[:, slot_map[j], :])
        in_maps.append({
            "seqt": seqT_blocked,
            "w_in": np.ascontiguousarray(w_in_j),
            "w_s": np.ascontiguousarray(w_s_jp),
            "biasb": np.ascontiguousarray(bias_b),
            "boutb": np.ascontiguousarray(bout_b),
            "ident": ident,
            "identr": ident.copy(),
            "h0t": h0t_p,
            "c0": np.ascontiguousarray(mem_c0[:, sl]),
        })
    return in_maps


def get_runner():
    if "runner" not in _CACHE:
        nc = _build_program()
        _CACHE["runner"] = _Runner(nc, replicated_names={"seqt"})
    return _CACHE["runner"]


def _slot_map_ok(pmaps):
    """pmaps: [8, 8, 4] per-core probe dumps. Returns (ok, learned_map)."""
    learned = []
    for j in range(NC_):
        row = [int(round(float(pmaps[j][k][0]))) for k in range(NC_)]
        learned.append(row)
    default = [[j ^ k for k in range(NC_)] for j in range(NC_)]
    return learned == default, learned


def kernel(seq, rel_labels, W_in, b_in, W_s, b_s, W_out, b_out, mem_h0, mem_c0):
    if not _labels_are_identity(rel_labels):
        return _np_reference(seq, rel_labels, W_in, b_in, W_s, b_s,
                             W_out, b_out, mem_h0, mem_c0)
    r = get_runner()
    in_maps = _prep_inputs(seq, W_in, b_in, W_s, b_s, W_out, b_out,
                           mem_h0, mem_c0)
    args = r.stage(in_maps)
    outs = r.run(args)
    return r.results(outs, core=0)["out"]
